# revision 3
# baseline (speedup 1.0000x reference)
"""Trainium2 Bass kernel for nn_MatrixSkipgram (embedding_lookup).

out[b] = ctx[X_context[b]] . (functor[X_functor[b]].reshape(E,E) @ noun[X_argument[b]])

Strategy (8 NeuronCores, data-parallel over batch):
  - Shard the 8192-element batch into 8 shards of 1024; replicate the three
    embedding tables on every core.
  - Per core, process 8 tiles of 128 batch elements (one per SBUF partition).
    For each tile, indirect-DMA-gather the 128 functor rows (40KB each), the
    128 noun rows and 128 context rows.
  - Compute per tile on the vector engine:
      * fused custom DVE op: prefix-scan of M[b,k]*argB[b,k] along the free
        dim (one pass, 10k elements); segment ends give the per-row matvec
        partial sums; an Abel-summation against the context vector folds the
        segment-diff and the final dot product into one tiny fused
        multiply+reduce.
  - One [128, 8] store per core; host reassembles the [8192] output.
"""

import os
import sys

import numpy as np

if "/opt/trn_rl_repo" not in sys.path:
    sys.path.insert(0, "/opt/trn_rl_repo")

NOUN_VOCAB = 50000
FUNC_VOCAB = 10000
CTX_VOCAB = 50000
EMBED = 100
BATCH = 8192
N_CORES = 8
SHARD = BATCH // N_CORES  # 1024
P = 128
N_TILES = SHARD // P  # 8

# compute path: "scan" (fused custom DVE op) or "baseline" (stock ops)
COMPUTE_PATH = os.environ.get("MSG_COMPUTE_PATH", "scan")

_cache = {}


def _register_mac_scan():
    """Register a custom DVE op: out[p,k] = cumsum_k(in0[p,k] * in1[p,k]).

    Registered at runtime (appended to dve_ops.OPS) so kernel.py stays
    self-contained; the per-NEFF DVE table is generated from OPS at compile
    time.
    """
    import concourse.dve_ops as dve_ops
    from concourse.dve_ops import OPS, DveOp
    from concourse.dve_spec import AluOp, Spec, Src0, Src1, _has_src1, lower, scan
    from concourse.dve_uop import DveOpSpec

    name = "MAC_SCAN_EMB"
    for o in OPS:
        if o.name == name:
            return o

    def _ref(in0, in1, s0, s1, imm2):
        p0 = in0.reshape(in0.shape[0], -1).astype(np.float32)
        p1 = np.broadcast_to(in1, in0.shape).reshape(in0.shape[0], -1)
        return np.cumsum(p0 * p1, axis=-1, dtype=np.float32).reshape(in0.shape)

    spec = Spec(body=scan(AluOp.ADD, Src0 * Src1), reference=_ref)
    row = max(dve_ops._SUB_OPCODE_FOR_NAME.values()) + 1
    assert row < 0x20
    shas = {}
    for ver in ("v3", "v4"):
        s = DveOpSpec(name=name, opcode=row, uops=lower(spec, ver=ver), rd1_en=_has_src1(spec))
        shas[ver] = s.sha(ver)
    dve_ops._SUB_OPCODE_FOR_NAME[name] = row
    op = DveOp(name, spec, subdim=False, uops_sha=shas)
    OPS.append(op)
    dve_ops.CUSTOM_DVE_SPECS[name] = spec
    return op


def _build(compute_path):
    import concourse.bacc as bacc
    import concourse.bass as bass
    import concourse.mybir as mybir
    from concourse.tile import TileContext

    f32 = mybir.dt.float32
    i32 = mybir.dt.int32
    mult = mybir.AluOpType.mult

    mac_op = _register_mac_scan() if compute_path == "scan" else None

    nc = bacc.Bacc(trn_type="TRN2", target_bir_lowering=False, debug=False)
    idx = nc.declare_dram_parameter("idx", [P, 3 * N_TILES], i32, isOutput=False)
    noun = nc.declare_dram_parameter("noun", [NOUN_VOCAB, EMBED], f32, isOutput=False)
    func = nc.declare_dram_parameter("func", [FUNC_VOCAB, EMBED * EMBED], f32, isOutput=False)
    ctxt = nc.declare_dram_parameter("ctxt", [CTX_VOCAB, EMBED], f32, isOutput=False)
    out = nc.declare_dram_parameter("out", [P, N_TILES], f32, isOutput=True)

    with TileContext(nc) as tc:
        with (
            tc.tile_pool(name="mpool", bufs=3) as mpool,
            tc.tile_pool(name="spool", bufs=3) as spool,
            tc.tile_pool(name="cpool", bufs=1) as cpool,
        ):
            idx_t = cpool.tile([P, 3 * N_TILES], i32)
            nc.sync.dma_start(out=idx_t[:], in_=idx[:])
            res = cpool.tile([P, N_TILES], f32)

            for t in range(N_TILES):
                arg = spool.tile([P, EMBED], f32, tag="arg")
                ctx_v = spool.tile([P, EMBED], f32, tag="ctx")
                M = mpool.tile([P, EMBED * EMBED], f32, tag="M")
                nc.gpsimd.indirect_dma_start(
                    out=arg[:],
                    out_offset=None,
                    in_=noun[:],
                    in_offset=bass.IndirectOffsetOnAxis(ap=idx_t[:, t : t + 1], axis=0),
                )
                nc.gpsimd.indirect_dma_start(
                    out=M[:],
                    out_offset=None,
                    in_=func[:],
                    in_offset=bass.IndirectOffsetOnAxis(
                        ap=idx_t[:, N_TILES + t : N_TILES + t + 1], axis=0
                    ),
                )
                nc.gpsimd.indirect_dma_start(
                    out=ctx_v[:],
                    out_offset=None,
                    in_=ctxt[:],
                    in_offset=bass.IndirectOffsetOnAxis(
                        ap=idx_t[:, 2 * N_TILES + t : 2 * N_TILES + t + 1], axis=0
                    ),
                )

                M3 = M[:].rearrange("p (i j) -> p i j", j=EMBED)
                argB = arg[:].unsqueeze(1).broadcast_to([P, EMBED, EMBED])

                if compute_path == "scan":
                    # One full-rate pass: pref[b,k] = cumsum_k(M[b,k] * argB[b,k]).
                    nc.vector._custom_dve(mac_op, out=M[:], in0=M[:], in1=argB)
                    # Segment ends e[b,i] = pref[b, i*E + E-1]; matvec row i is
                    # e[b,i]-e[b,i-1].  Abel summation: sum_i ctx[b,i]*(e_i-e_{i-1})
                    # = sum_i g[b,i]*e[b,i], g_i = ctx_i-ctx_{i+1} (g_{E-1}=ctx_{E-1}).
                    e = M3[:, :, EMBED - 1 : EMBED].squeeze(2)
                    g = spool.tile([P, EMBED], f32, tag="g")
                    junk = spool.tile([P, EMBED], f32, tag="junk")
                    nc.vector.tensor_tensor(
                        out=g[:, 0 : EMBED - 1],
                        in0=ctx_v[:, 0 : EMBED - 1],
                        in1=ctx_v[:, 1:EMBED],
                        op=mybir.AluOpType.subtract,
                    )
                    nc.vector.tensor_copy(
                        out=g[:, EMBED - 1 : EMBED], in_=ctx_v[:, EMBED - 1 : EMBED]
                    )
                    nc.vector.scalar_tensor_tensor(
                        out=junk[:],
                        in0=e,
                        scalar=1.0,
                        in1=g[:],
                        op0=mult,
                        op1=mult,
                        accum_out=res[:, t : t + 1],
                    )
                else:
                    nc.vector.tensor_tensor(out=M3, in0=M3, in1=argB, op=mult)
                    fa = spool.tile([P, EMBED], f32, tag="fa")
                    junk = spool.tile([P, EMBED], f32, tag="junk")
                    nc.vector.tensor_reduce(
                        out=fa[:], in_=M3, axis=mybir.AxisListType.X, op=mybir.AluOpType.add
                    )
                    nc.vector.scalar_tensor_tensor(
                        out=junk[:],
                        in0=fa[:],
                        scalar=1.0,
                        in1=ctx_v[:],
                        op0=mult,
                        op1=mult,
                        accum_out=res[:, t : t + 1],
                    )

            nc.sync.dma_start(out=out[:], in_=res[:])
    nc.finalize()
    return nc


def _get_nc():
    key = COMPUTE_PATH
    if key not in _cache:
        _cache[key] = _build(key)
    return _cache[key]


def _prep_inputs(X_argument, X_functor, X_context, noun_matrix, functor_table, context_table):
    noun = np.ascontiguousarray(np.asarray(noun_matrix, dtype=np.float32))
    func = np.ascontiguousarray(np.asarray(functor_table, dtype=np.float32))
    ctxt = np.ascontiguousarray(np.asarray(context_table, dtype=np.float32))
    in_maps = []
    for k in range(N_CORES):
        sl = slice(k * SHARD, (k + 1) * SHARD)
        cols = []
        for v in (X_argument, X_functor, X_context):
            vk = np.asarray(v, dtype=np.int32)[sl]
            cols.append(vk.reshape(N_TILES, P).T)  # [128, 8]: row p, col t = vk[t*128+p]
        idx = np.ascontiguousarray(np.concatenate(cols, axis=1))  # [128, 24]
        in_maps.append({"idx": idx, "noun": noun, "func": func, "ctxt": ctxt})
    return in_maps


def run(inputs, trace=False, **kw):
    """Run the SPMD kernel; returns (full_output [8192] f32, BassKernelResults)."""
    from concourse.bass_utils import run_bass_kernel_spmd

    nc = _get_nc()
    in_maps = _prep_inputs(**inputs)
    r = run_bass_kernel_spmd(nc, in_maps, list(range(N_CORES)), trace=trace, **kw)
    shards = [r.results[k]["out"].T.reshape(SHARD) for k in range(N_CORES)]
    return np.concatenate(shards).astype(np.float32), r


def kernel(**inputs) -> np.ndarray:
    out, _ = run(inputs, trace=False)
    return out


if __name__ == "__main__":
    rng = np.random.default_rng(0)
    inputs = {
        "X_argument": rng.integers(0, NOUN_VOCAB, BATCH).astype(np.int32),
        "X_functor": rng.integers(0, FUNC_VOCAB, BATCH).astype(np.int32),
        "X_context": rng.integers(0, CTX_VOCAB, BATCH).astype(np.int32),
        "noun_matrix": rng.standard_normal((NOUN_VOCAB, EMBED), dtype=np.float32),
        "functor_table": rng.standard_normal((FUNC_VOCAB, EMBED * EMBED), dtype=np.float32),
        "context_table": rng.standard_normal((CTX_VOCAB, EMBED), dtype=np.float32),
    }
    out = kernel(**inputs)
    print(out.shape, out.dtype, out[:4])
